# revision 1
# baseline (speedup 1.0000x reference)
"""Involution2d (B=8, C=256, H=W=56, K=7, G=16, reduction=4) on 8 TRN2 NeuronCores.

Sharding: spatial over H (7 output rows per core, 3-row halos), full batch
on-chip.  Involution partition layout = (group g, batch b) = 128 partitions:
per-pixel kernel maps broadcast across the 16 channels of their group via a
free-dim step-0 AP, tap shifts are free-dim offsets into padded x rows.

This environment charges ~1us per non-contiguous run in DMA/compute APs and
~80us latency per dependency-chained op, so v2 minimizes short runs and
dependency depth:
  - kernel-generation matmul chunks are tap-major (112 cols = 16 groups x 7
    taps of one kh row); the partition rearrange (g,k),(b,p) -> (g,b),(k,p)
    is a contiguous DRAM write plus per-group strided reads split across
    both HWDGE queues.
  - involution taps multiply over the full padded width (runs of 448) and
    accumulate via a depth-3 tree per kh row, fp32 master accumulator.
  - output is stored as one flat [128, 7168] DMA; the host unpacks.
"""

import os
import sys

import numpy as np

for _p in ("/opt/trn_rl_repo",):
    if os.path.isdir(_p) and _p not in sys.path:
        sys.path.insert(0, _p)

import concourse.bacc as bacc
import concourse.mybir as mybir
from concourse.tile import TileContext
from concourse.bass_utils import run_bass_kernel_spmd

# Problem constants (hardcoded per the task contract).
B, C, H, W = 8, 256, 56, 56
G, K, PAD = 16, 7, 3
CPG = C // G            # 16 channels per group
KK = K * K              # 49 taps
CR = 64                 # reduced channels
NCORES = 8
HS = H // NCORES        # 7 rows per core
HALO = PAD
HP = HS + 2 * HALO      # 13 padded rows
LPAD = 4                # left W-pad (even -> bf16 4B alignment)
WP = 64                 # padded row width: 4 + 56 + 4
NPIX = HS * WP          # 448 padded pixels per sample slab
NALLP = B * NPIX        # 3584 matmul moving dim
CROW = HP * WP          # 832 x elems per (c') row
XFLAT = CPG * CROW      # 13312 flat x elems per partition
XPAD = 14336            # x tile free size (lead pad + data + slack)
XOFF = 8                # x data offset inside the tile
NF = CPG * NPIX         # 7168 involution elems per partition

F32 = mybir.dt.float32
BF16 = mybir.dt.bfloat16

MCHUNK = G * K          # 112 ker rows per chunk = one kh row, all groups
NCHUNKS = K             # 7 chunks
NHALF = NALLP // 2      # 1792


def _build(reps=1):
    nc = bacc.Bacc(trn_type="TRN2")

    xs = nc.dram_tensor("xs", [B, C, HP, WP], F32, kind="ExternalInput").ap()
    xsmm = nc.dram_tensor("xsmm", [C, NALLP], F32, kind="ExternalInput").ap()
    w1t = nc.dram_tensor("w1t", [C, CR], F32, kind="ExternalInput").ap()
    b1 = nc.dram_tensor("b1", [CR, 1], F32, kind="ExternalInput").ap()
    # tap-major permuted: column j*112 + g*7 + kk = w_span row (g*49+j*7+kk)
    w2t = nc.dram_tensor("w2t", [CR, G * KK], F32, kind="ExternalInput").ap()
    b2 = nc.dram_tensor("b2", [MCHUNK, NCHUNKS], F32, kind="ExternalInput").ap()
    out = nc.dram_tensor("out", [128, NF], F32, kind="ExternalOutput").ap()
    kscratch = nc.dram_tensor(
        "kscratch", [reps, NCHUNKS, MCHUNK, NALLP], BF16
    ).ap()

    with TileContext(nc) as tc:
        with (
            tc.tile_pool(name="const", bufs=1) as cpool,
            tc.tile_pool(name="xp", bufs=1) as xpool,
            tc.tile_pool(name="work", bufs=1) as wpool,
            tc.tile_pool(name="stage", bufs=2) as spool,
            tc.tile_pool(name="pp", bufs=4) as prodpool,
            tc.tile_pool(name="psum", bufs=2, space="PSUM") as ppool,
        ):
            # ---------------- weights / biases ----------------
            lhsT1 = []
            for i in range(2):
                t = cpool.tile([128, CR], BF16, tag=f"w1_{i}", name=f"w1_{i}")
                nc.gpsimd.dma_start(out=t[:, :], in_=w1t[i * 128:(i + 1) * 128, :])
                lhsT1.append(t)
            w2all = cpool.tile([CR, G * KK], BF16, tag="w2", name="w2all")
            nc.gpsimd.dma_start(out=w2all[:, :], in_=w2t[:, :])
            lhsT2 = [w2all[:, j * MCHUNK:(j + 1) * MCHUNK] for j in range(NCHUNKS)]
            b2all = cpool.tile([MCHUNK, NCHUNKS], F32, tag="b2", name="b2all")
            nc.sync.dma_start(out=b2all[:, :], in_=b2[:, :])
            b2t = [b2all[:, j:j + 1] for j in range(NCHUNKS)]
            b1t = cpool.tile([CR, 1], F32, tag="b1", name="b1")
            nc.sync.dma_start(out=b1t[:, :], in_=b1[:, :])

            # ---------------- x loads ----------------
            x_even = xpool.tile([128, XPAD], BF16, tag="xe", name="x_even")
            xs_g = xs.rearrange("b (g c) h w -> g b (c h w)", g=G)
            nc.vector.memset(x_even[:, :], 0.0)
            nc.gpsimd.dma_start(out=x_even[:, XOFF:XOFF + XFLAT], in_=xs_g)

            xmm = []
            for i in range(2):
                t = spool.tile([128, NALLP], BF16, tag="kst", bufs=3,
                               name=f"xmm_{i}")
                nc.gpsimd.dma_start(
                    out=t[:, :], in_=xsmm[i * 128:(i + 1) * 128, :]
                )
                xmm.append(t)

            z_sb = wpool.tile([CR, NALLP], BF16, tag="z", name="z_sb")
            acc = wpool.tile([128, NF], F32, tag="acc", name="acc")

            def nsplits(lo, hi):
                r = []
                n0 = lo
                while n0 < hi:
                    r.append((n0, min(hi, n0 + 512)))
                    n0 += 512
                return r

            for rep in range(reps):
                # ---------------- z = w_reduce @ x ----------------
                for half in range(2):
                    lo, hi = half * NHALF, (half + 1) * NHALF
                    psum_z = ppool.tile(
                        [CR, NHALF], F32, tag="ps", name=f"psz{rep}_{half}"
                    )
                    for i in range(2):
                        for (a, b_) in nsplits(lo, hi):
                            nc.tensor.matmul(
                                out=psum_z[:, a - lo:b_ - lo],
                                lhsT=lhsT1[i][:, :],
                                rhs=xmm[i][:, a:b_],
                                start=(i == 0),
                                stop=(i == 1),
                            )
                    nc.scalar.add(z_sb[:, lo:hi], psum_z[:, :], b1t[:, 0:1])

                # ---------------- ker chunks (one kh row each) -------------
                ktaps = []
                for j in range(NCHUNKS):
                    kst = spool.tile(
                        [MCHUNK, NALLP], BF16, tag="kst", bufs=3,
                        name=f"kst{rep}_{j}"
                    )
                    for half in range(2):
                        lo, hi = half * NHALF, (half + 1) * NHALF
                        psum_k = ppool.tile(
                            [MCHUNK, NHALF], F32, tag="ps",
                            name=f"psk{rep}_{j}_{half}",
                        )
                        for (a, b_) in nsplits(lo, hi):
                            nc.tensor.matmul(
                                out=psum_k[:, a - lo:b_ - lo],
                                lhsT=lhsT2[j],
                                rhs=z_sb[:, a:b_],
                                start=True,
                                stop=True,
                            )
                        nc.scalar.add(
                            kst[:, lo:hi], psum_k[:, :], b2t[j]
                        )
                    # contiguous spill, then per-group strided reads that
                    # land ker in [(g,b), (kk,p)] partition layout
                    weng = nc.sync if j % 2 == 0 else nc.scalar
                    weng.dma_start(out=kscratch[rep, j, :, :], in_=kst[:, :])
                    ktap = spool.tile(
                        [128, K, NPIX], BF16, tag="ktap", bufs=3,
                        name=f"ktap{rep}_{j}"
                    )
                    for g in range(G):
                        reng = nc.sync if g % 2 == 0 else nc.scalar
                        reng.dma_start(
                            out=ktap[g * B:(g + 1) * B, :, :],
                            in_=kscratch[rep, j, g * K:(g + 1) * K].rearrange(
                                "kk (b p) -> b kk p", b=B
                            ),
                        )
                    ktaps.append(ktap)

                # ---------------- involution on DVE ----------------
                with nc.allow_low_precision("involution bf16 tree partials"):
                    for kh in range(K):
                        ktap = ktaps[kh]

                        def mul(kw, nm):
                            base = XOFF + kh * WP + (kw - PAD)
                            xin = x_even[:, base:base + XFLAT].rearrange(
                                "p (c r) -> p c r", c=CPG
                            )[:, :, 0:NPIX]
                            kin = ktap[:, kw:kw + 1, :].broadcast_to(
                                (128, CPG, NPIX)
                            )
                            m = prodpool.tile(
                                [128, CPG, NPIX], BF16, tag="m", name=nm
                            )
                            nc.vector.tensor_mul(m[:, :, :], xin, kin)
                            return m.rearrange("p c r -> p (c r)")

                        def tadd(x0, x1, nm):
                            t = prodpool.tile([128, NF], BF16, tag="m", name=nm)
                            nc.vector.tensor_add(t[:, :], x0[:, :], x1[:, :])
                            return t

                        # 4-slot-safe order: peak 4 live tiles in tag "m"
                        p = f"{rep}_{kh}"
                        m0 = mul(0, f"m0_{p}")
                        m1 = mul(1, f"m1_{p}")
                        a0 = tadd(m0, m1, f"a0_{p}")
                        m2 = mul(2, f"m2_{p}")
                        m3 = mul(3, f"m3_{p}")
                        a1 = tadd(m2, m3, f"a1_{p}")
                        ra = tadd(a0, a1, f"ra_{p}")
                        m4 = mul(4, f"m4_{p}")
                        m5 = mul(5, f"m5_{p}")
                        a2 = tadd(m4, m5, f"a2_{p}")
                        m6 = mul(6, f"m6_{p}")
                        rb = tadd(a2, m6, f"rb_{p}")
                        row = tadd(ra, rb, f"row_{p}")
                        if kh == 0:
                            nc.vector.tensor_copy(acc[:, :], row[:, :])
                        else:
                            nc.vector.tensor_add(acc[:, :], acc[:, :], row[:, :])

                # ---------------- store ----------------
                nc.scalar.dma_start(out=out, in_=acc[:, :])

    return nc


_CACHE = {}


def _get_program(reps=1):
    if reps not in _CACHE:
        nc = _build(reps)
        nc.compile()
        _CACHE[reps] = nc
    return _CACHE[reps]


def _make_inputs(x, w_reduce, b_reduce, w_span, b_span):
    x = np.ascontiguousarray(np.asarray(x, dtype=np.float32))
    w1t = np.ascontiguousarray(np.asarray(w_reduce, np.float32).T)
    b1 = np.ascontiguousarray(np.asarray(b_reduce, np.float32).reshape(-1, 1))
    # permute w_span rows tap-major: chunk j gets (g, kk) -> row g*49+j*7+kk
    w_span = np.asarray(w_span, np.float32)
    b_span = np.asarray(b_span, np.float32)
    perm = np.empty(G * KK, np.int64)
    idx = 0
    for j in range(NCHUNKS):
        for g in range(G):
            for kk in range(K):
                perm[idx] = g * KK + j * K + kk
                idx += 1
    w2t = np.ascontiguousarray(w_span[perm].T)
    b2 = np.ascontiguousarray(b_span[perm].reshape(NCHUNKS, MCHUNK).T)
    in_maps = []
    for i in range(NCORES):
        h0 = i * HS - HALO
        sl = np.zeros((B, C, HP, WP), np.float32)
        s0, s1 = max(0, h0), min(H, h0 + HP)
        sl[:, :, s0 - h0:s1 - h0, LPAD:LPAD + W] = x[:, :, s0:s1, :]
        xsmm = np.ascontiguousarray(
            sl[:, :, HALO:HALO + HS, :].transpose(1, 0, 2, 3).reshape(C, NALLP)
        )
        in_maps.append({"xs": sl, "xsmm": xsmm, "w1t": w1t, "b1": b1,
                        "w2t": w2t, "b2": b2})
    return in_maps


def _unpack_out(arr):
    """[128, NF] fp32 -> [B, C, HS, W]"""
    a = arr.reshape(G, B, CPG, HS, WP)[:, :, :, :, LPAD:LPAD + W]
    return np.ascontiguousarray(a.transpose(1, 0, 2, 3, 4)).reshape(B, C, HS, W)


def kernel_with_results(x, w_reduce, b_reduce, w_span, b_span, trace=False, reps=1):
    in_maps = _make_inputs(x, w_reduce, b_reduce, w_span, b_span)
    nc = _get_program(reps)
    res = run_bass_kernel_spmd(nc, in_maps, list(range(NCORES)), trace=trace)
    full = np.concatenate(
        [_unpack_out(res.results[i]["out"]) for i in range(NCORES)], axis=2
    ).astype(np.float32)
    return full, res


def kernel(x, w_reduce, b_reduce, w_span, b_span):
    full, _ = kernel_with_results(x, w_reduce, b_reduce, w_span, b_span)
    return full



# revision 2
# speedup vs baseline: 1.8918x; 1.8918x over previous
"""Involution2d v3 (B=8, C=256, H=W=56, K=7, G=16, reduction=4) on 8 TRN2 cores.

Spatial shard over H (7 output rows/core, 3-row halos), full batch on-chip,
partition layout (g, b) = 128.  v3 vs the v2 baseline:
  - per kh chunk: 1 contiguous spill + 1 rearranging mega-DMA (896 runs)
    instead of 1 spill + 16 per-group strided reads (17 instructions);
  - involution uses multi-kw mega-ops: per kh 2 muls (even/odd kw groups)
    + 4 in-place tree adds + 1 fp32 acc op = 49 DVE ops/rep instead of 98;
  - xmm matmul inputs live in their own pool (v2 parked them in the
    rotating kst tag, serializing all chunk pipelines on one buffer).
"""

import os
import sys

import numpy as np

for _p in ("/opt/trn_rl_repo",):
    if os.path.isdir(_p) and _p not in sys.path:
        sys.path.insert(0, _p)

import concourse.bacc as bacc
import concourse.mybir as mybir
from concourse.ap import AP
from concourse.tile import TileContext
from concourse.bass_utils import run_bass_kernel_spmd

# Problem constants (hardcoded per the task contract).
B, C, H, W = 8, 256, 56, 56
G, K, PAD = 16, 7, 3
CPG = C // G            # 16 channels per group
KK = K * K              # 49 taps
CR = 64                 # reduced channels
NCORES = 8
HS = H // NCORES        # 7 rows per core
HALO = PAD
HP = HS + 2 * HALO      # 13 padded rows
LPAD = 4                # left W-pad
WP = 64                 # padded row width: 4 + 56 + 4
NPIX = HS * WP          # 448 padded pixels per sample slab
NALLP = B * NPIX        # 3584 matmul moving dim
CROW = HP * WP          # 832 x elems per channel row
XFLAT = CPG * CROW      # 13312 flat x elems per partition
XOFF = 9                # odd -> even-kw taps 4B-aligned (bigger mul gets 2x)
XPAD = XOFF + XFLAT + 7  # x tile free size
NF = CPG * NPIX         # 7168 involution elems per partition

F32 = mybir.dt.float32
BF16 = mybir.dt.bfloat16

MCHUNK = G * K          # 112 ker rows per chunk = one kh row, all groups
NCHUNKS = K             # 7 chunks
NHALF = NALLP // 2      # 1792
KWE = [0, 2, 4, 6]      # even kw taps (aligned -> 2x mode)
KWO = [1, 3, 5]         # odd kw taps (1x mode)


def _strided(tile, offset, dims):
    """Custom free-dim AP on an SBUF tile: dims = [(stride, size), ...]."""
    base = tile[:, :] if tile.ndim == 2 else tile.rearrange(
        "q a p -> q (a p)")[:, :]
    part = list(base.ap)[0]
    return AP(base.tensor, base.offset + offset,
              [list(part)] + [[s, n] for (s, n) in dims])


def _build(reps=1):
    nc = bacc.Bacc(trn_type="TRN2")

    xs = nc.dram_tensor("xs", [B, C, HP, WP], F32, kind="ExternalInput").ap()
    xsmm = nc.dram_tensor("xsmm", [C, NALLP], F32, kind="ExternalInput").ap()
    w1t = nc.dram_tensor("w1t", [C, CR], F32, kind="ExternalInput").ap()
    b1 = nc.dram_tensor("b1", [CR, 1], F32, kind="ExternalInput").ap()
    # tap-major permuted: column j*112 + g*7 + kw = w_span row (g*49+j*7+kw)
    w2t = nc.dram_tensor("w2t", [CR, G * KK], F32, kind="ExternalInput").ap()
    b2 = nc.dram_tensor("b2", [MCHUNK, NCHUNKS], F32, kind="ExternalInput").ap()
    out = nc.dram_tensor("out", [128, NF], F32, kind="ExternalOutput").ap()
    kscratch = nc.dram_tensor(
        "kscratch", [reps, NCHUNKS, MCHUNK, NALLP], BF16
    ).ap()

    with TileContext(nc) as tc:
        with (
            tc.tile_pool(name="const", bufs=1) as cpool,
            tc.tile_pool(name="xp", bufs=1) as xpool,
            tc.tile_pool(name="work", bufs=1) as wpool,
            tc.tile_pool(name="kst", bufs=2) as kpool,
            tc.tile_pool(name="ktap", bufs=2) as tpool,
            tc.tile_pool(name="psum", bufs=2, space="PSUM") as ppool,
        ):
            # ---------------- weights / biases ----------------
            lhsT1 = []
            for i in range(2):
                t = cpool.tile([128, CR], BF16, tag=f"w1_{i}", name=f"w1_{i}")
                nc.gpsimd.dma_start(out=t[:, :], in_=w1t[i * 128:(i + 1) * 128, :])
                lhsT1.append(t)
            w2all = cpool.tile([CR, G * KK], BF16, tag="w2", name="w2all")
            nc.gpsimd.dma_start(out=w2all[:, :], in_=w2t[:, :])
            lhsT2 = [w2all[:, j * MCHUNK:(j + 1) * MCHUNK] for j in range(NCHUNKS)]
            b2all = cpool.tile([MCHUNK, NCHUNKS], F32, tag="b2", name="b2all")
            nc.sync.dma_start(out=b2all[:, :], in_=b2[:, :])
            b2t = [b2all[:, j:j + 1] for j in range(NCHUNKS)]
            b1t = cpool.tile([CR, 1], F32, tag="b1", name="b1")
            nc.sync.dma_start(out=b1t[:, :], in_=b1[:, :])

            # ---------------- x loads ----------------
            x_even = xpool.tile([128, XPAD], BF16, tag="xe", name="x_even")
            xs_g = xs.rearrange("b (g c) h w -> g b (c h w)", g=G)
            nc.vector.memset(x_even[:, :], 0.0)
            nc.gpsimd.dma_start(out=x_even[:, XOFF:XOFF + XFLAT], in_=xs_g)

            xmm = []
            for i in range(2):
                t = cpool.tile([128, NALLP], BF16, tag=f"xmm{i}", name=f"xmm_{i}")
                nc.gpsimd.dma_start(
                    out=t[:, :], in_=xsmm[i * 128:(i + 1) * 128, :]
                )
                xmm.append(t)

            z_sb = wpool.tile([CR, NALLP], BF16, tag="z", name="z_sb")
            acc = wpool.tile([128, NF], F32, tag="acc", name="acc")
            # products: even kw group [4, CPG, NPIX], odd kw group [3, ...]
            pe_t = wpool.tile([128, 4, CPG, NPIX], BF16, tag="pe", name="pe")
            po_t = wpool.tile([128, 3, CPG, NPIX], BF16, tag="po", name="po")

            def nsplits(lo, hi):
                r = []
                n0 = lo
                while n0 < hi:
                    r.append((n0, min(hi, n0 + 512)))
                    n0 += 512
                return r

            for rep in range(reps):
                # ---------------- z = w_reduce @ x ----------------
                for half in range(2):
                    lo, hi = half * NHALF, (half + 1) * NHALF
                    psum_z = ppool.tile(
                        [CR, NHALF], F32, tag="ps", name=f"psz{rep}_{half}"
                    )
                    for i in range(2):
                        for (a, b_) in nsplits(lo, hi):
                            nc.tensor.matmul(
                                out=psum_z[:, a - lo:b_ - lo],
                                lhsT=lhsT1[i][:, :],
                                rhs=xmm[i][:, a:b_],
                                start=(i == 0),
                                stop=(i == 1),
                            )
                    nc.scalar.add(z_sb[:, lo:hi], psum_z[:, :], b1t[:, 0:1])

                # ---------------- ker chunks (one kh row each) -------------
                ktaps = []
                for j in range(NCHUNKS):
                    kst = kpool.tile(
                        [MCHUNK, NALLP], BF16, tag="kst", name=f"kst{rep}_{j}"
                    )
                    for half in range(2):
                        lo, hi = half * NHALF, (half + 1) * NHALF
                        psum_k = ppool.tile(
                            [MCHUNK, NHALF], F32, tag="ps",
                            name=f"psk{rep}_{j}_{half}",
                        )
                        for (a, b_) in nsplits(lo, hi):
                            nc.tensor.matmul(
                                out=psum_k[:, a - lo:b_ - lo],
                                lhsT=lhsT2[j],
                                rhs=z_sb[:, a:b_],
                                start=True,
                                stop=True,
                            )
                        nc.scalar.add(
                            kst[:, lo:hi], psum_k[:, :], b2t[j]
                        )
                    # rearrange (g kw),(b p) -> (g b),(kw p): contiguous spill
                    # + 7 per-kw strided reads (3-dim DRAM APs)
                    ktap = tpool.tile(
                        [128, K, NPIX], BF16, tag="ktap", name=f"ktap{rep}_{j}"
                    )
                    weng = nc.sync if j % 2 == 0 else nc.scalar
                    weng.dma_start(out=kscratch[rep, j, :, :], in_=kst[:, :])
                    ks_base = kscratch[rep, j]
                    for kw in range(K):
                        eng = nc.sync if kw % 2 == 0 else nc.scalar
                        src = AP(
                            ks_base.tensor,
                            ks_base.offset + kw * NALLP,
                            [[K * NALLP, G], [NPIX, B], [1, NPIX]],
                        )
                        eng.dma_start(out=ktap[:, kw, :], in_=src)
                    ktaps.append(ktap)

                # ---------------- involution on DVE ----------------
                with nc.allow_low_precision("involution bf16 tree partials"):
                    for kh in range(K):
                        ktap = ktaps[kh]
                        base = XOFF + kh * WP - PAD
                        # x element for (kw, c, p): base + kw + c*CROW + p
                        xin_e = _strided(
                            x_even, base + KWE[0],
                            [(2, 4), (CROW, CPG), (1, NPIX)],
                        )
                        xin_o = _strided(
                            x_even, base + KWO[0],
                            [(2, 3), (CROW, CPG), (1, NPIX)],
                        )
                        # ktap value for (kw, c, p): kw*NPIX + p (c bcast)
                        kin_e = _strided(
                            ktap, KWE[0] * NPIX,
                            [(2 * NPIX, 4), (0, CPG), (1, NPIX)],
                        )
                        kin_o = _strided(
                            ktap, KWO[0] * NPIX,
                            [(2 * NPIX, 3), (0, CPG), (1, NPIX)],
                        )
                        nc.vector.tensor_mul(pe_t[:, :, :, :], xin_e, kin_e)
                        nc.vector.tensor_mul(po_t[:, :, :, :], xin_o, kin_o)
                        # in-place tree: pe[0:3] += po; pe[0] += pe[1];
                        # pe[0] += pe[2]; pe[0] += pe[3]; acc (+)= pe[0]
                        nc.vector.tensor_add(
                            pe_t[:, 0:3], pe_t[:, 0:3], po_t[:, :, :, :]
                        )
                        nc.vector.tensor_add(
                            pe_t[:, 0:1], pe_t[:, 0:1], pe_t[:, 1:2]
                        )
                        nc.vector.tensor_add(
                            pe_t[:, 0:1], pe_t[:, 0:1], pe_t[:, 2:3]
                        )
                        nc.vector.tensor_add(
                            pe_t[:, 0:1], pe_t[:, 0:1], pe_t[:, 3:4]
                        )
                        flat = pe_t[:, 0:1].rearrange("q a c p -> q (a c p)")
                        if kh == 0:
                            nc.vector.tensor_copy(acc[:, :], flat)
                        else:
                            nc.vector.tensor_add(acc[:, :], acc[:, :], flat)

                # ---------------- store ----------------
                nc.sync.dma_start(out=out, in_=acc[:, :])

    return nc


_CACHE = {}


def _get_program(reps=1):
    if reps not in _CACHE:
        nc = _build(reps)
        nc.compile()
        _CACHE[reps] = nc
    return _CACHE[reps]


def _make_inputs(x, w_reduce, b_reduce, w_span, b_span):
    x = np.ascontiguousarray(np.asarray(x, dtype=np.float32))
    w1t = np.ascontiguousarray(np.asarray(w_reduce, np.float32).T)
    b1 = np.ascontiguousarray(np.asarray(b_reduce, np.float32).reshape(-1, 1))
    # permute w_span rows tap-major: chunk j gets (g, kw) -> row g*49+j*7+kw
    w_span = np.asarray(w_span, np.float32)
    b_span = np.asarray(b_span, np.float32)
    perm = np.empty(G * KK, np.int64)
    idx = 0
    for j in range(NCHUNKS):
        for g in range(G):
            for kw in range(K):
                perm[idx] = g * KK + j * K + kw
                idx += 1
    w2t = np.ascontiguousarray(w_span[perm].T)
    b2 = np.ascontiguousarray(b_span[perm].reshape(NCHUNKS, MCHUNK).T)
    in_maps = []
    for i in range(NCORES):
        h0 = i * HS - HALO
        sl = np.zeros((B, C, HP, WP), np.float32)
        s0, s1 = max(0, h0), min(H, h0 + HP)
        sl[:, :, s0 - h0:s1 - h0, LPAD:LPAD + W] = x[:, :, s0:s1, :]
        xsmm = np.ascontiguousarray(
            sl[:, :, HALO:HALO + HS, :].transpose(1, 0, 2, 3).reshape(C, NALLP)
        )
        in_maps.append({"xs": sl, "xsmm": xsmm, "w1t": w1t, "b1": b1,
                        "w2t": w2t, "b2": b2})
    return in_maps


def _unpack_out(arr):
    """[128, NF] fp32 -> [B, C, HS, W]"""
    a = arr.reshape(G, B, CPG, HS, WP)[:, :, :, :, LPAD:LPAD + W]
    return np.ascontiguousarray(a.transpose(1, 0, 2, 3, 4)).reshape(B, C, HS, W)


def kernel_with_results(x, w_reduce, b_reduce, w_span, b_span, trace=False, reps=1):
    in_maps = _make_inputs(x, w_reduce, b_reduce, w_span, b_span)
    nc = _get_program(reps)
    res = run_bass_kernel_spmd(nc, in_maps, list(range(NCORES)), trace=trace)
    full = np.concatenate(
        [_unpack_out(res.results[i]["out"]) for i in range(NCORES)], axis=2
    ).astype(np.float32)
    return full, res


def kernel(x, w_reduce, b_reduce, w_span, b_span):
    full, _ = kernel_with_results(x, w_reduce, b_reduce, w_span, b_span)
    return full
